# revision 11
# baseline (speedup 1.0000x reference)
"""Multi-head attention Trainium2 kernel (B=4, S=2048, E=1024, H=16, D=64).

Sharding: head-parallel x data-parallel. Core c owns heads {2c, 2c+1} for all
4 batches -> 8 (batch, head) jobs per core, no cross-core communication.

All matmuls run in bf16 (fp32r triggers the TRN2 fp32 power throttle that
caps PE utilization at 50%); PSUM accumulation stays fp32.

PE-array tiling: the score matmuls contract over D=64, so two chunks run
concurrently in the two 64-row halves of the 128x128 array (row tiling via
base partitions 0/64). The q/k projections (K=64, M=64) run as a 2x2 tile
grid (4 concurrent matmuls), v projections as row-tiled pairs. For this,
x / W / biases are duplicated across both partition halves.

Per (batch, head) job on device:
  qT = (Wq/8)^T @ xT + bq/8           [128, 2048]   (dup halves; bias via DVE)
  kT = Wk^T @ xT + bk                 [128, 2048]   (dup halves)
  v  = xT^T @ Wv                      per 128-chunk [128, 64]  (no bias)
  scoresT[k, q] = kT_chunk^T @ qT     [128, 512] x2 concurrent (chunk pair)
  attnT = exp(scoresT)                ACT reads PSUM [128, 1024], writes bf16
  outT[65, q] += v_aug_chunk^T @ attnT   accumulated over 16 k-chunks in PSUM;
                                          row 64 = sum_k attnT = softmax denom
                                          (ones col in v_aug via memset)
Host side: shard/unshard reshapes, out = num/denom + bv (v-bias folds out
exactly because sum_k attn = denom), bias/scale folding.
"""

import numpy as np
import ml_dtypes

import concourse.bass as bass
import concourse.mybir as mybir
import concourse.tile as tile
from concourse.bass_utils import run_bass_kernel_spmd

F32 = mybir.dt.float32
BF16 = mybir.dt.bfloat16

B, S, E, H = 4, 2048, 1024, 16
D = E // H            # 64
NCORES = 8
HPC = H // NCORES     # heads per core = 2
PAIRS = B * HPC       # jobs per core = 8
NQ = 512              # q-group width
NG = S // NQ          # 4 q groups
KC = S // 128         # 16 k chunks of 128
KP = KC // 2          # 8 chunk pairs
VW = 2 * (D + 1)      # 130: v pair block [cA, onesA, cB, onesB]


def _patched_drain_and_barrier(self, tick_clock, wait_clock):
    # This walrus build rejects >1 sync-wait on a Drain (CTRL) instruction.
    # Collect the TileContext-exit waits on individual NOPs instead.
    nc = self.nc
    collector = nc.sync.nop(nofuse=True)
    wait_clock.add_sem_waits(
        collector.ins, tile.ScopedClock({None: tick_clock.global_clock})
    )
    si = collector.ins.sync_info
    if si is not None and len(si.on_wait) > 1:
        waits = list(si.on_wait)
        collector.ins.sync_info = mybir.SyncInfo(
            on_wait=[waits[0]], on_update=list(si.on_update)
        )
        for w in waits[1:]:
            n2 = nc.sync.nop(nofuse=True)
            n2.ins.sync_info = mybir.SyncInfo(on_wait=[w], on_update=[])
    nc.sync.drain()
    nc.all_engine_barrier()
    popped = nc._tile_sem_poison_stack.pop()
    assert popped is self._sem_poison
    nc.clear_and_free_semaphores(list(self.sems.allocated().values()))
    nc.all_engine_barrier()


tile.TileContext._drain_and_barrier = _patched_drain_and_barrier

_MAX_WAITS = 1


def _split_excess_waits(nc):
    """This walrus build allows at most one sync-wait per instruction; hoist
    extra waits onto NOPs inserted immediately before, on the same engine."""
    n = 0
    for f in nc.m.functions:
        for bb in f.blocks:
            new_insts = []
            for inst in bb.instructions:
                si = inst.sync_info
                if si is not None and len(si.on_wait) > _MAX_WAITS:
                    waits = list(si.on_wait)
                    for w in waits[:-_MAX_WAITS]:
                        nop = mybir.InstNoOp(
                            name=f"waitnop-{n}",
                            engine=inst.engine,
                            ins=[],
                            outs=[],
                            sync_info=mybir.SyncInfo(on_wait=[w], on_update=[]),
                            bass_nofuse=True,
                        )
                        n += 1
                        new_insts.append(nop)
                    inst.sync_info = mybir.SyncInfo(
                        on_wait=waits[-_MAX_WAITS:],
                        on_update=list(si.on_update),
                    )
                new_insts.append(inst)
            bb.instructions = new_insts


_NC_CACHE = {}


def build_nc():
    if "nc" in _NC_CACHE:
        return _NC_CACHE["nc"]
    nc = bass.Bass()
    xt = nc.dram_tensor("xt", [PAIRS, 128, S], BF16, kind="ExternalInput")
    w2 = nc.dram_tensor("w2", [HPC, 128, D], BF16, kind="ExternalInput")
    wv2 = nc.dram_tensor("wv2", [HPC, 128, D], BF16, kind="ExternalInput")
    bq2 = nc.dram_tensor("bq2", [HPC, 128, 1], F32, kind="ExternalInput")
    bk2 = nc.dram_tensor("bk2", [HPC, 128, 1], F32, kind="ExternalInput")
    out = nc.dram_tensor("out", [PAIRS, D + 1, S], F32, kind="ExternalOutput")

    with tile.TileContext(nc) as tc:
        with (
            tc.tile_pool(name="sb", bufs=2) as sb,
            tc.tile_pool(name="at", bufs=3) as atp,
            tc.tile_pool(name="wp", bufs=1) as wp,
            tc.tile_pool(name="sp", bufs=3, space="PSUM") as sp,
            tc.tile_pool(name="op", bufs=2, space="PSUM") as op,
        ):
            # weights/biases resident for the whole kernel (tiny)
            w_t = {}
            for nm, dram, dt in (("w2", w2, BF16), ("wv2", wv2, BF16),
                                 ("bq2", bq2, F32), ("bk2", bk2, F32)):
                for jj in range(HPC):
                    shp = [128, D] if dt == BF16 else [128, 1]
                    t = wp.tile(shp, dt, tag=f"{nm}{jj}")
                    nc.sync.dma_start(t[:], dram[jj])
                    w_t[nm, jj] = t

            def load_pair(p):
                t = sb.tile([128, S], BF16, tag="xt")
                nc.sync.dma_start(t[:], xt[p])
                return t

            def proj_qk(xt_t, jj, qt2, kt2, g):
                # plain q/k matmuls into partitions 0-63; the duplicate copy
                # into partitions 64-127 goes via SBUF->SBUF DMA (the only
                # partition-crossing path; DMA engines are otherwise idle)
                sl = bass.ts(g, NQ)
                ps = sp.tile([128, 2 * NQ], F32, tag="s")
                wj = w_t["w2", jj]
                nc.tensor.matmul(ps[0:D, 0:NQ], wj[0:D, :], xt_t[0:D, sl],
                                 start=True, stop=True)
                nc.tensor.matmul(ps[0:D, NQ:], wj[D:128, :], xt_t[D:128, sl],
                                 start=True, stop=True)
                nc.vector.tensor_scalar_add(qt2[0:D, sl], ps[0:D, 0:NQ],
                                            w_t["bq2", jj][0:D])
                nc.vector.tensor_scalar_add(kt2[0:D, sl], ps[0:D, NQ:],
                                            w_t["bk2", jj][0:D])
                nc.gpsimd.dma_start(qt2[D:128, sl], qt2[0:D, sl])
                nc.gpsimd.dma_start(kt2[D:128, sl], kt2[0:D, sl])

            def proj_v_pair(xt_t, jj, v_t, i):
                # row-tiled pair (rows 0-63 / 64-127), different PSUM banks
                psv = sp.tile([128, 2 * NQ], F32, tag="s")
                wvj = w_t["wv2", jj]
                nc.tensor.matmul(psv[:, 0:D], xt_t[0:D, bass.ts(2 * i, 128)],
                                 wvj[0:D, :], start=True, stop=True)
                nc.tensor.matmul(psv[:, NQ:NQ + D],
                                 xt_t[D:128, bass.ts(2 * i + 1, 128)],
                                 wvj[D:128, :], start=True, stop=True)
                vb = i * VW
                nc.vector.tensor_copy(v_t[:, vb:vb + D], psv[:, 0:D])
                nc.vector.tensor_copy(v_t[:, vb + D + 1:vb + 2 * D + 1],
                                      psv[:, NQ:NQ + D])

            cur = load_pair(0)
            for p in range(PAIRS):
                j = p % HPC
                xt_t = cur

                qt2 = sb.tile([128, S], BF16, tag="qt")
                kt2 = sb.tile([128, S], BF16, tag="kt")
                v_t = sb.tile([128, KP * VW], BF16, tag="v")
                nc.vector.memset(v_t[:], 1.0)

                proj_qk(xt_t, j, qt2, kt2, 0)
                proj_v_pair(xt_t, j, v_t, 0)

                if p + 1 < PAIRS:
                    cur = load_pair(p + 1)

                for g in range(NG):
                    qsl = bass.ts(g, NQ)
                    out_ps = op.tile([D + 1, NQ], F32, tag="out")
                    pend = None
                    for i in range(KP):
                        sps = sp.tile([128, 2 * NQ], F32, tag="s")
                        nc.tensor.matmul(sps[:, 0:NQ],
                                         kt2[0:D, bass.ts(2 * i, 128)],
                                         qt2[0:D, qsl],
                                         start=True, stop=True)
                        nc.tensor.matmul(sps[:, NQ:],
                                         kt2[D:128, bass.ts(2 * i + 1, 128)],
                                         qt2[D:128, qsl],
                                         start=True, stop=True)
                        at = atp.tile([128, 2 * NQ], BF16, tag="attn")
                        nc.scalar.activation(at[:], sps[:],
                                             mybir.ActivationFunctionType.Exp)
                        if g == 0:
                            if i < NG - 1:
                                proj_qk(xt_t, j, qt2, kt2, i + 1)
                            if i < KP - 1:
                                proj_v_pair(xt_t, j, v_t, i + 1)
                        if pend is not None:
                            pat, pi = pend
                            vb = pi * VW
                            nc.tensor.matmul(out_ps[:], v_t[:, vb:vb + D + 1],
                                             pat[:, 0:NQ],
                                             start=(pi == 0), stop=False)
                            nc.tensor.matmul(out_ps[:],
                                             v_t[:, vb + D + 1:vb + VW],
                                             pat[:, NQ:],
                                             start=False, stop=False)
                        pend = (at, i)
                    pat, pi = pend
                    vb = pi * VW
                    nc.tensor.matmul(out_ps[:], v_t[:, vb:vb + D + 1],
                                     pat[:, 0:NQ], start=False, stop=False)
                    nc.tensor.matmul(out_ps[:], v_t[:, vb + D + 1:vb + VW],
                                     pat[:, NQ:], start=False, stop=True)

                    # un-normalized [numerator; denominator] to HBM via an
                    # SBUF staging copy (DMA cannot read PSUM); the host
                    # divides rows 0:64 by row 64 during unshard
                    o_t = sb.tile([D + 1, NQ], F32, tag="o")
                    nc.vector.tensor_copy(o_t[:], out_ps[:])
                    nc.gpsimd.dma_start(out[p, :, qsl], o_t[:])

    _split_excess_waits(nc)
    _NC_CACHE["nc"] = nc
    return nc


def _prep_inputs(sequences, Wq, bq, Wk, bk, Wv, bv):
    s = 1.0 / np.sqrt(D)
    x = np.ascontiguousarray(np.asarray(sequences, dtype=np.float32))
    xh = x.reshape(B, S, H, D).transpose(2, 0, 3, 1)      # [H, B, D, S]
    x2 = np.concatenate([xh, xh], axis=2)                 # [H, B, 128, S]
    x2 = x2.astype(ml_dtypes.bfloat16)

    Wq = np.asarray(Wq, np.float32) * s
    Wk = np.asarray(Wk, np.float32)
    Wv = np.asarray(Wv, np.float32)
    bqs = np.asarray(bq, np.float32) * s
    bks = np.asarray(bk, np.float32)

    w2 = np.concatenate([Wq, Wk], axis=1).astype(ml_dtypes.bfloat16)
    wv2 = np.concatenate([Wv, Wv], axis=1).astype(ml_dtypes.bfloat16)
    bq2 = np.concatenate([bqs, bqs], axis=1)[:, :, None].astype(np.float32)
    bk2 = np.concatenate([bks, bks], axis=1)[:, :, None].astype(np.float32)

    in_maps = []
    for c in range(NCORES):
        xt_core = np.ascontiguousarray(np.stack(
            [x2[HPC * c + jj, b] for b in range(B) for jj in range(HPC)]))
        hs = slice(HPC * c, HPC * (c + 1))
        in_maps.append({
            "xt": xt_core,
            "w2": np.ascontiguousarray(w2[hs]),
            "wv2": np.ascontiguousarray(wv2[hs]),
            "bq2": np.ascontiguousarray(bq2[hs]),
            "bk2": np.ascontiguousarray(bk2[hs]),
        })
    return in_maps


def _assemble(results, bv):
    bv = np.asarray(bv, np.float32)
    out = np.empty((B, S, E), np.float32)
    for c in range(NCORES):
        r = results[c]["out"]                              # [8, 65, 2048]
        for b in range(B):
            for jj in range(HPC):
                h = HPC * c + jj
                rp = r[HPC * b + jj]
                out[b, :, h * D:(h + 1) * D] = (
                    (rp[:D] / rp[D:D + 1]).T + bv[h][None, :])
    return out


def run(trace=False, **inputs):
    nc = build_nc()
    in_maps = _prep_inputs(**inputs)
    res = run_bass_kernel_spmd(nc, in_maps, list(range(NCORES)), trace=trace)
    return _assemble(res.results, inputs["bv"]), res


def kernel(**inputs):
    out, _ = run(trace=False, **inputs)
    return out
